# revision 26
# baseline (speedup 1.0000x reference)
"""Trainium2 Bass kernel for nn_Attention_4363686773373.

Sigmoid attention with magnitude-preserving (weight-normalized) projections.

Sharding: data-parallel over (batch, T-half) -> 8 shards on 8 NeuronCores.
Each core computes q for its 1024 tokens and k,v for the full 2048 tokens of
its batch (k/v recomputed on both cores of a batch; no collectives).

Per-core dataflow (all heavy matmuls in bf16 with fp32 PSUM accumulation):
  W: row-normalize qkv_w / out_w on device; bounce bf16 rows through DRAM and
     transpose with large DMA-xbar transposes -> wnT, ownT
  X: cast x to bf16 (gpsimd DRAM->DRAM cast DMA), large transposes -> xkvT;
     per-token ||x|| from f32 x -> mag
  A: qkv projection (natural [t,e] layout), q/k cosine-normalize along head_dim
     (free-dim reduce), bounce qn/kn through DRAM -> [head_dim, t] layout
  B: per head: scoresT = knT^T qnT (K=64 row-packed head pairs), sigmoid on the
     scalar engine (PSUM->SBUF bf16), attnT @ v accumulated over key blocks;
     per-pair PE-transposes bring attn-out back to natural layout
  C: normalize av per (token, head), scale by token magnitude, PE-transpose,
     out-projection.

DMA dispatch rings: sync = xbar transposes, scalar(ACT) = weight/x loads,
gpsimd(SWDGE) = DRAM scratch writes + casts + output stores.
"""

import math
from contextlib import ExitStack

import numpy as np

import concourse.bass as bass
import concourse.tile as tile
from concourse import bacc, mybir
from concourse.bass_utils import run_bass_kernel_spmd
from concourse.masks import make_identity

# Problem shapes (hardcoded per harness contract)
B, T, D, H = 4, 2048, 768, 12
HD = D // H  # 64
EPS = 1e-4
SIGMOID_GAIN = 1.8402
N_CORES = 8

F32 = mybir.dt.float32
BF16 = mybir.dt.bfloat16
AF = mybir.ActivationFunctionType
ALU = mybir.AluOpType
AX = mybir.AxisListType


def _ensure_axon_hooks():
    """This image's antenv lacks axon_hooks; reconstruct it so trace=True
    (NTFF profiling) works instead of crashing on import."""
    try:
        import antenv.axon_hooks  # noqa: F401
        return
    except ImportError:
        pass
    import sys
    import types
    try:
        import antenv
    except ImportError:
        return
    mod = types.ModuleType("antenv.axon_hooks")
    _hook = [None]
    mod.set_axon_ntff_profile_hook = lambda h: _hook.__setitem__(0, h)
    mod.get_axon_ntff_profile_hook = lambda: _hook[0]
    sys.modules["antenv.axon_hooks"] = mod
    antenv.axon_hooks = mod
    try:
        from trn_agent_boot.trn_boot import _ntff_profile_via_ctypes
        mod.set_axon_ntff_profile_hook(
            _ntff_profile_via_ctypes('/opt/axon/libaxon_pjrt.so'))
    except Exception:
        pass


_ensure_axon_hooks()

if __import__("os").environ.get("ANT_LDW_OPT") == "1":
    import concourse.bass_utils as _bu
    _orig_rc = _bu.run_command

    def _rc_ldw(argv, **kw):
        argv = ["--enable-ldw-opt=true" if a == "--enable-ldw-opt=false" else a
                for a in argv]
        return _orig_rc(argv, **kw)

    _bu.run_command = _rc_ldw


def _chunks(total, maxn=1024):
    out = []
    c0 = 0
    while c0 < total:
        cn = min(maxn, total - c0)
        out.append((c0, cn))
        c0 += cn
    return out


def build_program(nc, tc, ctx, Tt, Dm, Hl, groups):
    """Head-sharded per-core program: this core owns Hl heads of one batch
    row. Inputs are pre-sliced on the host so the program is SPMD-uniform:
    qkvw holds only this core's q/k/v weight rows ([3*Dl, Dm]); outw is
    column-permuted so this core's head dims are columns 0..Dl. The partial
    out-projection is pair-reduced with ReduceScatter; each core emits the
    final output for half the tokens."""
    keep = []  # keep tc.tile free-closures alive (GC would release the pools)

    def _tile(shape, dtype, name):
        t, free = tc.tile(shape, dtype, name=name)
        keep.append(free)
        return t, free

    tc._ant_keepalive = keep
    P = 128
    HDl = 64
    Dl = Hl * HDl             # this core's head-dim total (384)
    E3l = 3 * Dl
    DT = Dm // P              # contraction d-tiles over x features (6)
    DTl = Dl // P             # this core's d-tiles (3)
    PAIRS = Hl // 2
    TB = Tt // P              # token blocks (16)
    WEl = E3l // P            # qkv weight row tiles (9)
    Thalf = Tt // 2
    eps_av = EPS * math.sqrt(Tt) / SIGMOID_GAIN

    xkv = nc.dram_tensor("xkv", [Tt, Dm], F32, kind="ExternalInput").ap()
    qkvw = nc.dram_tensor("qkvw", [E3l, Dm], F32, kind="ExternalInput").ap()
    outw = nc.dram_tensor("outw", [Dm, Dm], F32, kind="ExternalInput").ap()
    y = nc.dram_tensor("y", [Thalf, Dm], F32, kind="ExternalOutput").ap()

    # ---------------- DRAM scratch ----------------
    dstk = ExitStack()
    dpool = dstk.enter_context(tc.tile_pool(name="dram", bufs=1, space="DRAM"))
    kn_dram = dpool.tile([Tt, Dl], BF16, name="kn_dram")
    qn_dram = dpool.tile([Tt, Dl], BF16, name="qn_dram")
    part_dram = dpool.tile([Tt, Dm], BF16, name="part_dram")
    rs_dram = dpool.tile([Thalf, Dm], BF16, name="rs_dram")

    # ---------------- persistent SBUF tensors ----------------
    knT, _ = _tile([P, PAIRS * Tt], BF16, "knT")     # [hd(2 heads), s]
    qnT, _ = _tile([P, PAIRS * Tt], BF16, "qnT")     # [hd(2 heads), t]
    vbig, _ = _tile([P, TB * Dl], BF16, "vbig")      # natural [s, e-local]
    mag8, _ = _tile([P, max(TB, 2)], F32, "mag8")    # sqrt(||x||^2*HD/D)
    ownT, _ = _tile([P, DTl * Dm], BF16, "ownT")     # out_w^T, own d rows
    avnat, _ = _tile([P, TB * Dl], BF16, "avnat")    # attn-out natural
    avt_big, _ = _tile([P, PAIRS * Tt], BF16, "avt_big")
    ident, _ = _tile([P, P], BF16, "ident")          # PE-transpose identity
    make_identity(nc, ident)

    # ---------------- phase W + X (scoped pools) ----------------
    wxa = ExitStack()
    wnT, free_wnT = _tile([P, DT * E3l], BF16, "wnT")
    xT, free_xT = _tile([P, DT * Tt], BF16, "xT")
    wstage = wxa.enter_context(tc.tile_pool(name="wstage", bufs=4))
    sqpool = wxa.enter_context(tc.tile_pool(name="sqpool", bufs=3))
    small = wxa.enter_context(tc.tile_pool(name="small", bufs=16))
    nstage = wxa.enter_context(tc.tile_pool(name="nstage", bufs=4))
    psA = wxa.enter_context(tc.tile_pool(name="psA", bufs=2, space="PSUM"))
    psW = wxa.enter_context(tc.tile_pool(name="psW", bufs=2, space="PSUM"))

    def pe_transpose_block(src_cols, dst_cols):
        ptw = psW.tile([P, P], BF16, name="ptw", tag="ptw")
        nc.tensor.transpose(ptw, src_cols, ident)
        nc.vector.tensor_copy(dst_cols, ptw)

    def normalize_tile(src_ap, we):
        """Load row-tile we, return bf16 rows/(||row||+eps) staging tile."""
        wst = wstage.tile([P, Dm], F32, name="wst", tag="wst")
        nc.scalar.dma_start(wst, src_ap[we * P:(we + 1) * P, :])
        wsq = sqpool.tile([P, Dm], BF16, name="wsq", tag="sq")
        ssw = small.tile([P, 1], F32, name="ssw", tag="s1")
        nc.scalar.activation(wsq, wst, AF.Square, accum_out=ssw)
        sw = small.tile([P, 1], F32, name="sw", tag="s1")
        nc.scalar.activation(sw, ssw, AF.Sqrt)
        swe = small.tile([P, 1], F32, name="swe", tag="s1")
        nc.vector.tensor_scalar_add(swe, sw, EPS)
        rw = small.tile([P, 1], F32, name="rw", tag="s1")
        nc.vector.reciprocal(rw, swe)
        wnb = nstage.tile([P, Dm], BF16, name="wnb", tag="nst")
        nc.vector.tensor_scalar_mul(wnb, wst, rw)
        return wnb

    def load_x(ti):
        """x token-block ti: magnitude, bf16 cast, PE-transpose into xT."""
        xst = wstage.tile([P, Dm], F32, name="xst", tag="wst")
        nc.sync.dma_start(xst, xkv[ti * P:(ti + 1) * P, :])
        xsq = sqpool.tile([P, Dm], BF16, name="xsq", tag="sq")
        ssx = small.tile([P, 1], F32, name="ssx", tag="s1")
        nc.scalar.activation(xsq, xst, AF.Square, accum_out=ssx)
        nc.scalar.activation(mag8[:, ti:ti + 1], ssx, AF.Sqrt,
                             scale=float(HDl) / float(Dm))
        xbf = nstage.tile([P, Dm], BF16, name="xbf", tag="nst")
        nc.vector.tensor_copy(xbf, xst)
        for dt in range(DT):
            pe_transpose_block(xbf[:, dt * P:(dt + 1) * P],
                               xT[:, dt * Tt + ti * P: dt * Tt + (ti + 1) * P])

    # interleave x blocks with qkv weight rows and out_w rows
    for i in range(max(TB, WEl + DT)):
        if i < TB:
            load_x(i)
        if i < WEl:
            wnb = normalize_tile(qkvw, i)
            for dt in range(DT):
                pe_transpose_block(
                    wnb[:, dt * P:(dt + 1) * P],
                    wnT[:, dt * E3l + i * P: dt * E3l + (i + 1) * P])
        elif i - WEl < DT:
            we = i - WEl
            wnb = normalize_tile(outw, we)
            for dtl in range(DTl):
                pe_transpose_block(
                    wnb[:, dtl * P:(dtl + 1) * P],
                    ownT[:, dtl * Dm + we * P: dtl * Dm + (we + 1) * P])

    # ---------------- phase A: qkv projection + q/k normalization ----------
    def qk_normalize(kraw, is_k):
        sqk = sqpool.tile([P, Dl], BF16, name="sqk", tag="sql")
        nc.vector.tensor_mul(sqk, kraw, kraw)
        ssk = small.tile([P, Hl], F32, name="ssk", tag="sh")
        nc.vector.tensor_reduce(ssk, sqk.rearrange("p (h d) -> p h d", h=Hl),
                                axis=AX.X, op=ALU.add)
        sk = small.tile([P, Hl], F32, name="sk", tag="sh")
        nc.scalar.activation(sk, ssk, AF.Sqrt)
        ske = small.tile([P, Hl], F32, name="ske", tag="sh")
        if is_k:
            nc.vector.tensor_scalar(ske, sk, EPS, 1.0 / math.sqrt(HDl),
                                    op0=ALU.add, op1=ALU.mult)
        else:
            nc.vector.tensor_scalar_add(ske, sk, EPS)
        rk = small.tile([P, Hl], F32, name="rk", tag="sh")
        nc.vector.reciprocal(rk, ske)
        knb = nstage.tile([P, Dl], BF16, name="knb", tag="nsl")
        nc.vector.tensor_tensor(
            knb.rearrange("p (h d) -> p h d", h=Hl),
            kraw.rearrange("p (h d) -> p h d", h=Hl),
            rk.broadcast_to([P, Hl, HDl]),
            op=ALU.mult)
        return knb

    KQ = max(TB // 4, 1)
    for ti in range(TB):
        ps = psA.tile([P, E3l], F32, name="psqkv", tag="ps")
        for dt in range(DT):
            lhs = xT[:, dt * Tt + ti * P: dt * Tt + (ti + 1) * P]
            for (c0, cn) in _chunks(E3l, 512):
                nc.tensor.matmul(ps[:, c0:c0 + cn], lhsT=lhs,
                                 rhs=wnT[:, dt * E3l + c0: dt * E3l + c0 + cn],
                                 start=(dt == 0), stop=(dt == DT - 1))
        qraw = sqpool.tile([P, Dl], BF16, name="qraw", tag="sql")
        nc.scalar.activation(qraw, ps[:, 0:Dl], AF.Copy)
        kraw = sqpool.tile([P, Dl], BF16, name="kraw", tag="sql")
        nc.scalar.activation(kraw, ps[:, Dl:2 * Dl], AF.Copy)
        nc.scalar.activation(vbig[:, ti * Dl:(ti + 1) * Dl], ps[:, 2 * Dl:3 * Dl],
                             AF.Copy)
        qnb = qk_normalize(qraw, False)
        nc.gpsimd.dma_start(qn_dram[ti * P:(ti + 1) * P, :], qnb)
        knb = qk_normalize(kraw, True)
        nc.gpsimd.dma_start(kn_dram[ti * P:(ti + 1) * P, :], knb)
        if ti % KQ == KQ - 1:
            h0 = (ti // KQ) * KQ * P
            hn = KQ * P
            for pr in range(PAIRS):
                nc.sync.dma_start_transpose(
                    knT[:, pr * Tt + h0: pr * Tt + h0 + hn],
                    kn_dram[h0:h0 + hn, pr * P:(pr + 1) * P])
                nc.sync.dma_start_transpose(
                    qnT[:, pr * Tt + h0: pr * Tt + h0 + hn],
                    qn_dram[h0:h0 + hn, pr * P:(pr + 1) * P])

    wxa.close()
    free_xT()
    free_wnT()

    # ---------------- phase B: scores -> sigmoid -> attn @ v ----------------
    bstk = ExitStack()
    psS = bstk.enter_context(tc.tile_pool(name="psS", bufs=3, space="PSUM"))
    psAV = bstk.enter_context(tc.tile_pool(name="psAV", bufs=1, space="PSUM"))
    attnp = bstk.enter_context(tc.tile_pool(name="attnp", bufs=4))

    THW = min(512, Thalf)
    NTH = Thalf // THW
    units = [(pr, hh, sb, th) for pr in range(PAIRS) for hh in range(2)
             for sb in range(TB) for th in range(NTH)]
    psav_by_key = {}
    pss_by_unit = {}

    def emit_scores(u):
        pr, hh, sb, th = u
        t0 = hh * Thalf + th * THW
        pss = psS.tile([P, 2 * THW], F32, name="pss", tag="pss")
        pss_by_unit[u] = pss
        for a in (0, 1):
            r0 = a * HDl
            nc.tensor.matmul(
                pss[:, a * THW:(a + 1) * THW],
                lhsT=knT[r0:r0 + HDl, pr * Tt + sb * P: pr * Tt + (sb + 1) * P],
                rhs=qnT[r0:r0 + HDl, pr * Tt + t0: pr * Tt + t0 + THW],
                start=True, stop=True)

    emit_scores(units[0])
    emit_scores(units[1])
    for i, u in enumerate(units):
        pr, hh, sb, th = u
        if i + 2 < len(units):
            emit_scores(units[i + 2])
        if sb == 0 and th == 0:
            psav_by_key[(pr, hh)] = psAV.tile([P, Thalf], F32, name="psav",
                                              tag="psav")
        psav = psav_by_key[(pr, hh)]
        pss = pss_by_unit.pop(u)
        attn = attnp.tile([P, 2 * THW], BF16, name="attn", tag="attn")
        nc.scalar.activation(attn, pss, AF.Sigmoid)
        for a in (0, 1):
            r0 = a * HDl
            nc.tensor.matmul(
                psav[r0:r0 + HDl, th * THW:(th + 1) * THW],
                lhsT=vbig[:, sb * Dl + pr * P + r0: sb * Dl + pr * P + r0 + HDl],
                rhs=attn[:, a * THW:(a + 1) * THW],
                start=(sb == 0), stop=(sb == TB - 1),
                skip_group_check=True)
        if sb == TB - 1 and th == NTH - 1:
            nc.vector.tensor_copy(
                avt_big[:, pr * Tt + hh * Thalf: pr * Tt + (hh + 1) * Thalf],
                psav)
    bstk.close()

    # ---------------- phase C: normalize + out-proj + pair-reduce ----------
    avnT, _ = _tile([P, DTl * Tt], BF16, "avnT")
    cstk = ExitStack()
    psO = cstk.enter_context(tc.tile_pool(name="psO", bufs=2, space="PSUM"))
    psT2 = cstk.enter_context(tc.tile_pool(name="psT2", bufs=4, space="PSUM"))
    sqc = cstk.enter_context(tc.tile_pool(name="sqc", bufs=3))
    smallc = cstk.enter_context(tc.tile_pool(name="smallc", bufs=16))
    avnp = cstk.enter_context(tc.tile_pool(name="avnp", bufs=3))
    ypool = cstk.enter_context(tc.tile_pool(name="ypool", bufs=2))

    NCH = max(1, min(4, Thalf // P))
    CR = Thalf // NCH
    assert CR % P == 0

    def part_row(tb):
        # partial rows stored chunk-interleaved so each ReduceScatter chunk
        # input ([2, CR, Dm] for the pair) is one contiguous block
        half = 0 if tb < TB // 2 else 1
        j = tb % (TB // 2)
        c = j // (CR // P)
        pos = j % (CR // P)
        return c * 2 * CR + half * CR + pos * P

    def c_avT(tb):
        for pr in range(PAIRS):
            ptt = psT2.tile([P, P], BF16, name="ptta", tag="ptt2")
            nc.tensor.transpose(
                ptt, avt_big[:, pr * Tt + tb * P: pr * Tt + (tb + 1) * P], ident)
            nc.scalar.activation(
                avnat[:, tb * Dl + pr * P: tb * Dl + (pr + 1) * P], ptt, AF.Copy)

    def c_norm(tb):
        src = avnat[:, tb * Dl:(tb + 1) * Dl]
        sqa = sqc.tile([P, Dl], BF16, name="sqa", tag="sqa")
        nc.vector.tensor_mul(sqa, src, src)
        ssa = smallc.tile([P, Hl], F32, name="ssa", tag="sh")
        nc.vector.tensor_reduce(ssa, sqa.rearrange("p (h d) -> p h d", h=Hl),
                                axis=AX.X, op=ALU.add)
        sa = smallc.tile([P, Hl], F32, name="sa", tag="sh")
        nc.scalar.activation(sa, ssa, AF.Sqrt)
        sae = smallc.tile([P, Hl], F32, name="sae", tag="sh")
        nc.vector.tensor_scalar_add(sae, sa, eps_av)
        ra = smallc.tile([P, Hl], F32, name="ra", tag="sh")
        nc.vector.reciprocal(ra, sae)
        g = smallc.tile([P, Hl], F32, name="g", tag="sh")
        nc.vector.tensor_scalar_mul(g, ra, mag8[:, tb:tb + 1])
        avn = avnp.tile([P, Dl], BF16, name="avn", tag="avn")
        nc.vector.tensor_tensor(
            avn.rearrange("p (h d) -> p h d", h=Hl),
            src.rearrange("p (h d) -> p h d", h=Hl),
            g.broadcast_to([P, Hl, HDl]),
            op=ALU.mult)
        for dtl in range(DTl):
            ptt = psT2.tile([P, P], BF16, name="ptt2", tag="ptt2")
            nc.tensor.transpose(ptt, avn[:, dtl * P:(dtl + 1) * P], ident)
            nc.vector.tensor_copy(
                avnT[:, dtl * Tt + tb * P: dtl * Tt + (tb + 1) * P], ptt)

    def c_proj(tb):
        pso = psO.tile([P, Dm], F32, name="pso", tag="pso")
        for dtl in range(DTl):
            lhs = avnT[:, dtl * Tt + tb * P: dtl * Tt + (tb + 1) * P]
            for (c0, cn) in _chunks(Dm, 512):
                nc.tensor.matmul(pso[:, c0:c0 + cn], lhsT=lhs,
                                 rhs=ownT[:, dtl * Dm + c0: dtl * Dm + c0 + cn],
                                 start=(dtl == 0), stop=(dtl == DTl - 1))
        ysb = ypool.tile([P, Dm], BF16, name="ysb", tag="ysb")
        nc.scalar.activation(ysb, pso, AF.Copy)
        r0 = part_row(tb)
        nc.gpsimd.dma_start(part_dram[r0:r0 + P, :], ysb)

    rs_done = 0

    def emit_rs(c):
        view3 = part_dram[c * 2 * CR:(c + 1) * 2 * CR, :].rearrange(
            "(g r) d -> g r d", g=2)
        nc.gpsimd.collective_compute(
            "ReduceScatter", ALU.add, replica_groups=groups,
            ins=[view3], outs=[rs_dram[c * CR:(c + 1) * CR, :]])
        nc.gpsimd.dma_start(y[c * CR:(c + 1) * CR, :],
                            rs_dram[c * CR:(c + 1) * CR, :])

    for tb in range(TB + 2):
        if tb < TB:
            c_avT(tb)
        if tb >= 1 and tb - 1 < TB:
            c_norm(tb - 1)
        if tb >= 2:
            c_proj(tb - 2)
            done = tb - 1  # out-proj blocks emitted so far
            while rs_done < NCH and done * P >= Thalf + (rs_done + 1) * CR:
                emit_rs(rs_done)
                rs_done += 1
    while rs_done < NCH:
        emit_rs(rs_done)
        rs_done += 1
    cstk.close()
    dstk.close()


def make_nc(Tt=T, Dm=D, Hl=H // 2, num_devices=N_CORES):
    nc = bacc.Bacc("TRN2", target_bir_lowering=False, debug=False,
                   num_devices=num_devices)
    groups = [[2 * i, 2 * i + 1] for i in range(num_devices // 2)]
    with ExitStack() as ctx:
        with tile.TileContext(nc) as tc:
            build_program(nc, tc, ctx, Tt, Dm, Hl, groups)
    nc.compile()
    return nc


_CACHED_NC = None


def _get_nc():
    global _CACHED_NC
    if _CACHED_NC is None:
        _CACHED_NC = make_nc()
    return _CACHED_NC


def shard_inputs(x, qkv_w, out_w, B_=B, Hn=H, Dm=D, n_cores=N_CORES):
    """Core c owns batch c//2 and head-group c%2 (Hn//2 heads)."""
    Dl = (Hn // 2) * (Dm // Hn)
    x = np.asarray(x, dtype=np.float32)
    qkv_w = np.asarray(qkv_w, dtype=np.float32)
    out_w = np.asarray(out_w, dtype=np.float32)
    in_maps = []
    for core in range(n_cores):
        b, hg = core // 2, core % 2
        r0 = hg * Dl
        qkv_sl = np.ascontiguousarray(np.concatenate(
            [qkv_w[k * Dm + r0:k * Dm + r0 + Dl] for k in range(3)], axis=0))
        outw_perm = np.ascontiguousarray(np.concatenate(
            [out_w[:, r0:r0 + Dl], out_w[:, (1 - hg) * Dl:(2 - hg) * Dl]],
            axis=1))
        in_maps.append({"xkv": np.ascontiguousarray(x[b]),
                        "qkvw": qkv_sl, "outw": outw_perm})
    return in_maps


def run(x, qkv_w, out_w, trace=False, trace_cores=None):
    nc = _get_nc()
    in_maps = shard_inputs(x, qkv_w, out_w)
    res = run_bass_kernel_spmd(nc, in_maps, list(range(N_CORES)),
                               trace=trace, trace_cores=trace_cores)
    Th = T // 2
    y = np.empty((B, T, D), np.float32)
    for core, r in enumerate(res.results):
        b, hg = core // 2, core % 2
        y[b, hg * Th:(hg + 1) * Th] = r["y"]
    return y, res


def kernel(x, qkv_w, out_w):
    y, _ = run(x, qkv_w, out_w, trace=False)
    return y


# revision 27
# speedup vs baseline: 1.2933x; 1.2933x over previous
"""Trainium2 Bass kernel for nn_Attention_4363686773373.

Sigmoid attention with magnitude-preserving (weight-normalized) projections.

Sharding: data-parallel over (batch, T-half) -> 8 shards on 8 NeuronCores.
Each core computes q for its 1024 tokens and k,v for the full 2048 tokens of
its batch (k/v recomputed on both cores of a batch; no collectives).

Per-core dataflow (all heavy matmuls in bf16 with fp32 PSUM accumulation):
  W: row-normalize qkv_w / out_w on device; bounce bf16 rows through DRAM and
     transpose with large DMA-xbar transposes -> wnT, ownT
  X: cast x to bf16 (gpsimd DRAM->DRAM cast DMA), large transposes -> xkvT;
     per-token ||x|| from f32 x -> mag
  A: qkv projection (natural [t,e] layout), q/k cosine-normalize along head_dim
     (free-dim reduce), bounce qn/kn through DRAM -> [head_dim, t] layout
  B: per head: scoresT = knT^T qnT (K=64 row-packed head pairs), sigmoid on the
     scalar engine (PSUM->SBUF bf16), attnT @ v accumulated over key blocks;
     per-pair PE-transposes bring attn-out back to natural layout
  C: normalize av per (token, head), scale by token magnitude, PE-transpose,
     out-projection.

DMA dispatch rings: sync = xbar transposes, scalar(ACT) = weight/x loads,
gpsimd(SWDGE) = DRAM scratch writes + casts + output stores.
"""

import math
from contextlib import ExitStack

import numpy as np

import concourse.bass as bass
import concourse.tile as tile
from concourse import bacc, mybir
from concourse.bass_utils import run_bass_kernel_spmd
from concourse.masks import make_identity

# Problem shapes (hardcoded per harness contract)
B, T, D, H = 4, 2048, 768, 12
HD = D // H  # 64
EPS = 1e-4
SIGMOID_GAIN = 1.8402
N_CORES = 8

F32 = mybir.dt.float32
BF16 = mybir.dt.bfloat16
AF = mybir.ActivationFunctionType
ALU = mybir.AluOpType
AX = mybir.AxisListType


def _ensure_axon_hooks():
    """This image's antenv lacks axon_hooks; reconstruct it so trace=True
    (NTFF profiling) works instead of crashing on import."""
    try:
        import antenv.axon_hooks  # noqa: F401
        return
    except ImportError:
        pass
    import sys
    import types
    try:
        import antenv
    except ImportError:
        return
    mod = types.ModuleType("antenv.axon_hooks")
    _hook = [None]
    mod.set_axon_ntff_profile_hook = lambda h: _hook.__setitem__(0, h)
    mod.get_axon_ntff_profile_hook = lambda: _hook[0]
    sys.modules["antenv.axon_hooks"] = mod
    antenv.axon_hooks = mod
    try:
        from trn_agent_boot.trn_boot import _ntff_profile_via_ctypes
        mod.set_axon_ntff_profile_hook(
            _ntff_profile_via_ctypes('/opt/axon/libaxon_pjrt.so'))
    except Exception:
        pass


_ensure_axon_hooks()

if __import__("os").environ.get("ANT_LDW_OPT") == "1":
    import concourse.bass_utils as _bu
    _orig_rc = _bu.run_command

    def _rc_ldw(argv, **kw):
        argv = ["--enable-ldw-opt=true" if a == "--enable-ldw-opt=false" else a
                for a in argv]
        return _orig_rc(argv, **kw)

    _bu.run_command = _rc_ldw


def _chunks(total, maxn=1024):
    out = []
    c0 = 0
    while c0 < total:
        cn = min(maxn, total - c0)
        out.append((c0, cn))
        c0 += cn
    return out


def build_program(nc, tc, ctx, Tq, Tkv, Dm, Hn):
    """Emit the per-core program. xkv rows are pre-ordered so the first Tq
    tokens are this core's query tokens (attention is permutation-invariant
    over the key axis)."""
    keep = []  # keep tc.tile free-closures alive (GC would release the pools)

    def _tile(shape, dtype, name):
        t, free = tc.tile(shape, dtype, name=name)
        keep.append(free)
        return t, free

    tc._ant_keepalive = keep
    P = 128
    HDl = 64
    assert Dm % P == 0 and Tq % P == 0 and Tkv % P == 0
    DT = Dm // P          # d-tiles
    E3 = 3 * Dm
    PAIRS = Hn // 2       # head pairs; pair = 128 contiguous features
    assert PAIRS * P == Dm and Hn * HDl == Dm
    TBq = Tq // P
    TBkv = Tkv // P
    WE = E3 // P          # qkv_w row tiles
    # eps seen by the post-attention normalize, after folding out the
    # gain/sqrt(T) prefactor (we accumulate raw attn@v).
    eps_av = EPS * math.sqrt(Tkv) / SIGMOID_GAIN

    xkv = nc.dram_tensor("xkv", [Tkv, Dm], F32, kind="ExternalInput").ap()
    qkvw = nc.dram_tensor("qkvw", [E3, Dm], F32, kind="ExternalInput").ap()
    outw = nc.dram_tensor("outw", [Dm, Dm], F32, kind="ExternalInput").ap()
    y = nc.dram_tensor("y", [Tq, Dm], F32, kind="ExternalOutput").ap()

    # ---------------- DRAM scratch ----------------
    dstk = ExitStack()
    dpool = dstk.enter_context(tc.tile_pool(name="dram", bufs=1, space="DRAM"))
    own_dram = dpool.tile([Dm, Dm], BF16, name="own_dram")
    kn_dram = dpool.tile([Tkv, Dm], BF16, name="kn_dram")
    qn_dram = dpool.tile([Tq, Dm], BF16, name="qn_dram")

    # ---------------- persistent SBUF tensors ----------------
    knT, _ = _tile([P, PAIRS * Tkv], BF16, "knT")    # [hd(2 heads), s]
    qnT, _ = _tile([P, PAIRS * Tq], BF16, "qnT")     # [hd(2 heads), t]
    vbig, _ = _tile([P, TBkv * Dm], BF16, "vbig")    # natural [s, e]
    mag8, _ = _tile([P, max(TBq, 2)], F32, "mag8")   # sqrt(||x||^2*HD/D)
    ownT, _ = _tile([P, DT * Dm], BF16, "ownT")      # out_w normalized^T
    avnat, _ = _tile([P, TBq * Dm], BF16, "avnat")   # attn-out natural
    ident, _ = _tile([P, P], BF16, "ident")          # PE-transpose identity
    make_identity(nc, ident)

    # ---------------- phase W + X + A (scoped) ----------------
    wxa = ExitStack()
    wnT, free_wnT = _tile([P, DT * E3], BF16, "wnT")
    xkvT, free_xkvT = _tile([P, DT * Tkv], BF16, "xkvT")
    wstage = wxa.enter_context(tc.tile_pool(name="wstage", bufs=6))
    sqpool = wxa.enter_context(tc.tile_pool(name="sqpool", bufs=4))
    small = wxa.enter_context(tc.tile_pool(name="small", bufs=16))
    nstage = wxa.enter_context(tc.tile_pool(name="nstage", bufs=6))
    psA = wxa.enter_context(tc.tile_pool(name="psA", bufs=2, space="PSUM"))
    psW = wxa.enter_context(tc.tile_pool(name="psW", bufs=2, space="PSUM"))

    def pe_transpose_cols(src, dst_big, cols, stride, base):
        """PE-transpose src [P, DT*P] column blocks into dst_big where block
        dt lands at dst_big[:, dt*stride + base : +cols]."""
        for dt in range(DT):
            ptw = psW.tile([P, P], BF16, name="ptw", tag="ptw")
            nc.tensor.transpose(ptw, src[:, dt * P:(dt + 1) * P], ident)
            nc.vector.tensor_copy(
                dst_big[:, dt * stride + base: dt * stride + base + cols], ptw)

    def normalize_w(we):
        """qkv_w row-tile we -> bf16 rows/(||row||+eps), PE-transposed into
        wnT."""
        wst = wstage.tile([P, Dm], F32, name="wst", tag="wst")
        nc.scalar.dma_start(wst, qkvw[we * P:(we + 1) * P, :])
        wsq = sqpool.tile([P, Dm], BF16, name="wsq", tag="sq")
        ssw = small.tile([P, 1], F32, name="ssw", tag="s1")
        nc.scalar.activation(wsq, wst, AF.Square, accum_out=ssw)
        sw = small.tile([P, 1], F32, name="sw", tag="s1")
        nc.scalar.activation(sw, ssw, AF.Sqrt)
        swe = small.tile([P, 1], F32, name="swe", tag="s1")
        nc.vector.tensor_scalar_add(swe, sw, EPS)
        rw = small.tile([P, 1], F32, name="rw", tag="s1")
        nc.vector.reciprocal(rw, swe)
        wnb = nstage.tile([P, Dm], BF16, name="wnb", tag="nst")
        nc.vector.tensor_scalar_mul(wnb, wst, rw)
        pe_transpose_cols(wnb, wnT, P, E3, we * P)

    def load_x(ti):
        """x token-block ti: magnitude, bf16 cast, PE-transpose into xkvT."""
        xst = wstage.tile([P, Dm], F32, name="xst", tag="wst")
        nc.sync.dma_start(xst, xkv[ti * P:(ti + 1) * P, :])
        if ti < TBq:
            xsq = sqpool.tile([P, Dm], BF16, name="xsq", tag="sq")
            ssx = small.tile([P, 1], F32, name="ssx", tag="s1")
            nc.scalar.activation(xsq, xst, AF.Square, accum_out=ssx)
            nc.scalar.activation(mag8[:, ti:ti + 1], ssx, AF.Sqrt,
                                 scale=float(HDl) / float(Dm))
        xbf = nstage.tile([P, Dm], BF16, name="xbf", tag="nst")
        nc.vector.tensor_copy(xbf, xst)
        pe_transpose_cols(xbf, xkvT, P, Tkv, ti * P)

    # interleave x blocks with k/v weight rows (rows Dm..3Dm); phase A's kv
    # loop needs all kv weight tiles + per-ti x tiles
    for i in range(max(TBkv, 2 * DT)):
        if i < TBkv:
            load_x(i)
        if i < 2 * DT:
            normalize_w(DT + i)
    for we in range(DT):     # q weight rows last (q loop runs after kv loop)
        normalize_w(we)

    # out-projection weights: normalize -> DRAM bounce -> xbar transpose
    # (only needed by phase C; uses idle DMA capacity during A/B)
    for we in range(DT):
        wst = wstage.tile([P, Dm], F32, name="wso", tag="wst")
        nc.scalar.dma_start(wst, outw[we * P:(we + 1) * P, :])
        wsq = sqpool.tile([P, Dm], BF16, name="wsqo", tag="sq")
        ssw = small.tile([P, 1], F32, name="sswo", tag="s1")
        nc.scalar.activation(wsq, wst, AF.Square, accum_out=ssw)
        sw = small.tile([P, 1], F32, name="swo", tag="s1")
        nc.scalar.activation(sw, ssw, AF.Sqrt)
        swe = small.tile([P, 1], F32, name="sweo", tag="s1")
        nc.vector.tensor_scalar_add(swe, sw, EPS)
        rw = small.tile([P, 1], F32, name="rwo", tag="s1")
        nc.vector.reciprocal(rw, swe)
        wnb = nstage.tile([P, Dm], BF16, name="wnbo", tag="nst")
        nc.vector.tensor_scalar_mul(wnb, wst, rw)
        nc.gpsimd.dma_start(own_dram[we * P:(we + 1) * P, :], wnb)
    for dt in range(DT):
        nc.sync.dma_start_transpose(
            ownT[:, dt * Dm:(dt + 1) * Dm],
            own_dram[:, dt * P:(dt + 1) * P])

    # qkv projection + q/k normalization, natural layout
    def qk_normalize(kraw, is_k):
        """kraw: SBUF bf16 [P, Dm] raw q or k; returns normalized bf16 tile."""
        sqk = sqpool.tile([P, Dm], BF16, name="sqk", tag="sq")
        nc.vector.tensor_mul(sqk, kraw, kraw)
        ssk = small.tile([P, Hn], F32, name="ssk", tag="sh")
        nc.vector.tensor_reduce(ssk, sqk.rearrange("p (h d) -> p h d", h=Hn),
                                axis=AX.X, op=ALU.add)
        sk = small.tile([P, Hn], F32, name="sk", tag="sh")
        nc.scalar.activation(sk, ssk, AF.Sqrt)
        ske = small.tile([P, Hn], F32, name="ske", tag="sh")
        if is_k:
            # fold the 1/sqrt(HD) score scale into k: sqrt(HD)/(||k||+eps)
            nc.vector.tensor_scalar(ske, sk, EPS, 1.0 / math.sqrt(HDl),
                                    op0=ALU.add, op1=ALU.mult)
        else:
            nc.vector.tensor_scalar_add(ske, sk, EPS)
        rk = small.tile([P, Hn], F32, name="rk", tag="sh")
        nc.vector.reciprocal(rk, ske)
        knb = nstage.tile([P, Dm], BF16, name="knb", tag="nst")
        nc.vector.tensor_tensor(
            knb.rearrange("p (h d) -> p h d", h=Hn),
            kraw.rearrange("p (h d) -> p h d", h=Hn),
            rk.broadcast_to([P, Hn, HDl]),
            op=ALU.mult)
        return knb

    def emit_q(ti):
        # q for this core's token blocks (first TBq blocks of xkv)
        ps = psA.tile([P, Dm], F32, name="psq", tag="ps")
        for dt in range(DT):
            lhs = xkvT[:, dt * Tkv + ti * P: dt * Tkv + (ti + 1) * P]
            for (c0, cn) in _chunks(Dm, 512):
                nc.tensor.matmul(ps[:, c0:c0 + cn], lhsT=lhs,
                                 rhs=wnT[:, dt * E3 + c0: dt * E3 + c0 + cn],
                                 start=(dt == 0), stop=(dt == DT - 1))
        qraw = sqpool.tile([P, Dm], BF16, name="qraw", tag="kraw")
        nc.scalar.activation(qraw, ps[:, 0:Dm], AF.Copy)
        qnb = qk_normalize(qraw, False)
        nc.gpsimd.dma_start(qn_dram[ti * P:(ti + 1) * P, :], qnb)
        QH = max(TBq // 2, 1)
        if ti % QH == QH - 1:
            h0 = (ti // QH) * QH * P
            hn = QH * P
            for pr in range(PAIRS):
                nc.sync.dma_start_transpose(
                    qnT[:, pr * Tq + h0: pr * Tq + h0 + hn],
                    qn_dram[h0:h0 + hn, pr * P:(pr + 1) * P])

    KQ = max(TBkv // 4, 1)
    qdone = 0
    for ti in range(TBkv):
        # k,v for every token block
        ps = psA.tile([P, 2 * Dm], F32, name="pskv", tag="ps")
        for dt in range(DT):
            lhs = xkvT[:, dt * Tkv + ti * P: dt * Tkv + (ti + 1) * P]
            for (c0, cn) in _chunks(2 * Dm, 512):
                nc.tensor.matmul(ps[:, c0:c0 + cn], lhsT=lhs,
                                 rhs=wnT[:, dt * E3 + Dm + c0: dt * E3 + Dm + c0 + cn],
                                 start=(dt == 0), stop=(dt == DT - 1))
        # evict PSUM quickly (frees the accumulation slot after two ACT copies)
        kraw = sqpool.tile([P, Dm], BF16, name="kraw", tag="kraw")
        nc.scalar.activation(kraw, ps[:, 0:Dm], AF.Copy)
        nc.scalar.activation(vbig[:, ti * Dm:(ti + 1) * Dm], ps[:, Dm:2 * Dm],
                             AF.Copy)
        knb = qk_normalize(kraw, True)
        nc.gpsimd.dma_start(kn_dram[ti * P:(ti + 1) * P, :], knb)
        if ti % KQ == KQ - 1:
            h0 = (ti // KQ) * KQ * P
            hn = KQ * P
            for pr in range(PAIRS):
                nc.sync.dma_start_transpose(
                    knT[:, pr * Tkv + h0: pr * Tkv + h0 + hn],
                    kn_dram[h0:h0 + hn, pr * P:(pr + 1) * P])
        # interleave q token-blocks so the PE stream stays dense into phase B
        qtarget = (ti + 1) * TBq // TBkv
        while qdone < qtarget:
            emit_q(qdone)
            qdone += 1

    wxa.close()
    free_xkvT()
    free_wnT()

    # ---------------- phase B: scores -> sigmoid -> attn @ v ----------------
    # Software-pipelined: scores for unit i+1 are issued to the PE before the
    # attn@v of unit i, so the PE works under each sigmoid instead of stalling
    # in FIFO order behind it. unit = (pair, key-block, head-in-pair).
    avt_big, _ = _tile([P, PAIRS * Tq], BF16, "avt_big")
    bstk = ExitStack()
    psS = bstk.enter_context(tc.tile_pool(name="psS", bufs=3, space="PSUM"))
    psAV = bstk.enter_context(tc.tile_pool(name="psAV", bufs=1, space="PSUM"))
    attnp = bstk.enter_context(tc.tile_pool(name="attnp", bufs=4))

    # unit = (pair, key-block, t-half). One [128, 1024] score tile holds BOTH
    # heads' [128, 512] score blocks side by side: the two K=64 matmuls are
    # emitted adjacently (concurrent in disjoint PE row groups), and ONE
    # FD=1024 sigmoid covers both heads.
    THW = min(512, Tq)
    TH = Tq // THW
    units = [(pr, sb, th) for pr in range(PAIRS) for sb in range(TBkv)
             for th in range(TH)]
    psav_by_pair = {}
    pss_by_unit = {}

    def emit_scores(u):
        pr, sb, th = u
        pss = psS.tile([P, 2 * THW], F32, name="pss", tag="pss")
        pss_by_unit[u] = pss
        for a in (0, 1):
            r0 = a * HDl
            nc.tensor.matmul(
                pss[:, a * THW:(a + 1) * THW],
                lhsT=knT[r0:r0 + HDl, pr * Tkv + sb * P: pr * Tkv + (sb + 1) * P],
                rhs=qnT[r0:r0 + HDl, pr * Tq + th * THW: pr * Tq + (th + 1) * THW],
                start=True, stop=True)

    emit_scores(units[0])
    emit_scores(units[1])
    for i, u in enumerate(units):
        pr, sb, th = u
        if i + 2 < len(units):
            emit_scores(units[i + 2])
        if sb == 0 and th == 0:
            psav_by_pair[pr] = psAV.tile([P, Tq], F32, name="psav", tag="psav")
        psav = psav_by_pair[pr]
        pss = pss_by_unit.pop(u)
        attn = attnp.tile([P, 2 * THW], BF16, name="attn", tag="attn")
        nc.scalar.activation(attn, pss, AF.Sigmoid)
        for a in (0, 1):
            r0 = a * HDl
            nc.tensor.matmul(
                psav[r0:r0 + HDl, th * THW:(th + 1) * THW],
                lhsT=vbig[:, sb * Dm + pr * P + r0: sb * Dm + pr * P + r0 + HDl],
                rhs=attn[:, a * THW:(a + 1) * THW],
                start=(sb == 0), stop=(sb == TBkv - 1),
                skip_group_check=True)
        if sb == TBkv - 1 and th == TH - 1:
            nc.vector.tensor_copy(avt_big[:, pr * Tq:(pr + 1) * Tq], psav)
    bstk.close()

    # ---------------- phase C: normalize + magnitude + out-proj ----------------
    avnT, _ = _tile([P, DT * Tq], BF16, "avnT")
    cstk = ExitStack()
    psO = cstk.enter_context(tc.tile_pool(name="psO", bufs=2, space="PSUM"))
    psT2 = cstk.enter_context(tc.tile_pool(name="psT2", bufs=4, space="PSUM"))
    sqc = cstk.enter_context(tc.tile_pool(name="sqc", bufs=3))
    smallc = cstk.enter_context(tc.tile_pool(name="smallc", bufs=16))
    avnp = cstk.enter_context(tc.tile_pool(name="avnp", bufs=3))
    ypool = cstk.enter_context(tc.tile_pool(name="ypool", bufs=2))

    def c_avT(tb):
        for pr in range(PAIRS):
            ptt = psT2.tile([P, P], BF16, name="ptta", tag="ptt2")
            nc.tensor.transpose(
                ptt, avt_big[:, pr * Tq + tb * P: pr * Tq + (tb + 1) * P], ident)
            nc.scalar.activation(
                avnat[:, tb * Dm + pr * P: tb * Dm + (pr + 1) * P], ptt, AF.Copy)

    def c_norm(tb):
        src = avnat[:, tb * Dm:(tb + 1) * Dm]
        sqa = sqc.tile([P, Dm], BF16, name="sqa", tag="sqa")
        nc.vector.tensor_mul(sqa, src, src)
        ssa = smallc.tile([P, Hn], F32, name="ssa", tag="sh")
        nc.vector.tensor_reduce(ssa, sqa.rearrange("p (h d) -> p h d", h=Hn),
                                axis=AX.X, op=ALU.add)
        sa = smallc.tile([P, Hn], F32, name="sa", tag="sh")
        nc.scalar.activation(sa, ssa, AF.Sqrt)
        sae = smallc.tile([P, Hn], F32, name="sae", tag="sh")
        nc.vector.tensor_scalar_add(sae, sa, eps_av)
        ra = smallc.tile([P, Hn], F32, name="ra", tag="sh")
        nc.vector.reciprocal(ra, sae)
        g = smallc.tile([P, Hn], F32, name="g", tag="sh")
        nc.vector.tensor_scalar_mul(g, ra, mag8[:, tb:tb + 1])
        avn = avnp.tile([P, Dm], BF16, name="avn", tag="avn")
        nc.vector.tensor_tensor(
            avn.rearrange("p (h d) -> p h d", h=Hn),
            src.rearrange("p (h d) -> p h d", h=Hn),
            g.broadcast_to([P, Hn, HDl]),
            op=ALU.mult)
        for dt in range(DT):
            ptt = psT2.tile([P, P], BF16, name="ptt2", tag="ptt2")
            nc.tensor.transpose(ptt, avn[:, dt * P:(dt + 1) * P], ident)
            nc.vector.tensor_copy(
                avnT[:, dt * Tq + tb * P: dt * Tq + (tb + 1) * P], ptt)

    def c_proj(tb):
        pso = psO.tile([P, Dm], F32, name="pso", tag="pso")
        for dt in range(DT):
            lhs = avnT[:, dt * Tq + tb * P: dt * Tq + (tb + 1) * P]
            for (c0, cn) in _chunks(Dm, 512):
                nc.tensor.matmul(pso[:, c0:c0 + cn], lhsT=lhs,
                                 rhs=ownT[:, dt * Dm + c0: dt * Dm + c0 + cn],
                                 start=(dt == 0), stop=(dt == DT - 1))
        ysb = ypool.tile([P, Dm], F32, name="ysb", tag="ysb")
        nc.scalar.activation(ysb, pso, AF.Copy)
        nc.gpsimd.dma_start(y[tb * P:(tb + 1) * P, :], ysb)

    for tb in range(TBq + 2):
        if tb < TBq:
            c_avT(tb)
        if tb >= 1 and tb - 1 < TBq:
            c_norm(tb - 1)
        if tb >= 2:
            c_proj(tb - 2)
    cstk.close()
    dstk.close()


def make_nc(Tq=T // 2, Tkv=T, Dm=D, Hn=H):
    nc = bacc.Bacc("TRN2", target_bir_lowering=False, debug=False,
                   num_devices=N_CORES)
    with ExitStack() as ctx:
        with tile.TileContext(nc) as tc:
            build_program(nc, tc, ctx, Tq, Tkv, Dm, Hn)
    nc.compile()
    return nc


_CACHED_NC = None


def _get_nc():
    global _CACHED_NC
    if _CACHED_NC is None:
        _CACHED_NC = make_nc()
    return _CACHED_NC


def _shard_inputs(x, qkv_w, out_w):
    Tq = T // 2
    x = np.asarray(x, dtype=np.float32)
    qkv_w = np.ascontiguousarray(np.asarray(qkv_w, dtype=np.float32))
    out_w = np.ascontiguousarray(np.asarray(out_w, dtype=np.float32))
    in_maps = []
    for core in range(N_CORES):
        b, half = core // 2, core % 2
        own = x[b, half * Tq:(half + 1) * Tq]
        other = x[b, (1 - half) * Tq:(2 - half) * Tq]
        xkv = np.ascontiguousarray(np.concatenate([own, other], axis=0))
        in_maps.append({"xkv": xkv, "qkvw": qkv_w, "outw": out_w})
    return in_maps


def run(x, qkv_w, out_w, trace=False, trace_cores=None):
    nc = _get_nc()
    in_maps = _shard_inputs(x, qkv_w, out_w)
    res = run_bass_kernel_spmd(nc, in_maps, list(range(N_CORES)),
                               trace=trace, trace_cores=trace_cores)
    Tq = T // 2
    y = np.empty((B, T, D), np.float32)
    for core, r in enumerate(res.results):
        b, half = core // 2, core % 2
        y[b, half * Tq:(half + 1) * Tq] = r["y"]
    return y, res


def kernel(x, qkv_w, out_w):
    y, _ = run(x, qkv_w, out_w, trace=False)
    return y


# revision 28
# speedup vs baseline: 1.3042x; 1.0084x over previous
"""Trainium2 Bass kernel for nn_Attention_4363686773373.

Sigmoid attention with magnitude-preserving (weight-normalized) projections.

Sharding: data-parallel over (batch, T-half) -> 8 shards on 8 NeuronCores.
Each core computes q for its 1024 tokens and k,v for the full 2048 tokens of
its batch (k/v recomputed on both cores of a batch; no collectives).

Per-core dataflow (all heavy matmuls in bf16 with fp32 PSUM accumulation):
  W: row-normalize qkv_w / out_w on device; bounce bf16 rows through DRAM and
     transpose with large DMA-xbar transposes -> wnT, ownT
  X: cast x to bf16 (gpsimd DRAM->DRAM cast DMA), large transposes -> xkvT;
     per-token ||x|| from f32 x -> mag
  A: qkv projection (natural [t,e] layout), q/k cosine-normalize along head_dim
     (free-dim reduce), bounce qn/kn through DRAM -> [head_dim, t] layout
  B: per head: scoresT = knT^T qnT (K=64 row-packed head pairs), sigmoid on the
     scalar engine (PSUM->SBUF bf16), attnT @ v accumulated over key blocks;
     per-pair PE-transposes bring attn-out back to natural layout
  C: normalize av per (token, head), scale by token magnitude, PE-transpose,
     out-projection.

DMA dispatch rings: sync = xbar transposes, scalar(ACT) = weight/x loads,
gpsimd(SWDGE) = DRAM scratch writes + casts + output stores.
"""

import math
from contextlib import ExitStack

import numpy as np

import concourse.bass as bass
import concourse.tile as tile
from concourse import bacc, mybir
from concourse.bass_utils import run_bass_kernel_spmd
from concourse.masks import make_identity

# Problem shapes (hardcoded per harness contract)
B, T, D, H = 4, 2048, 768, 12
HD = D // H  # 64
EPS = 1e-4
SIGMOID_GAIN = 1.8402
N_CORES = 8

F32 = mybir.dt.float32
BF16 = mybir.dt.bfloat16
AF = mybir.ActivationFunctionType
ALU = mybir.AluOpType
AX = mybir.AxisListType


def _ensure_axon_hooks():
    """This image's antenv lacks axon_hooks; reconstruct it so trace=True
    (NTFF profiling) works instead of crashing on import."""
    try:
        import antenv.axon_hooks  # noqa: F401
        return
    except ImportError:
        pass
    import sys
    import types
    try:
        import antenv
    except ImportError:
        return
    mod = types.ModuleType("antenv.axon_hooks")
    _hook = [None]
    mod.set_axon_ntff_profile_hook = lambda h: _hook.__setitem__(0, h)
    mod.get_axon_ntff_profile_hook = lambda: _hook[0]
    sys.modules["antenv.axon_hooks"] = mod
    antenv.axon_hooks = mod
    try:
        from trn_agent_boot.trn_boot import _ntff_profile_via_ctypes
        mod.set_axon_ntff_profile_hook(
            _ntff_profile_via_ctypes('/opt/axon/libaxon_pjrt.so'))
    except Exception:
        pass


_ensure_axon_hooks()

if __import__("os").environ.get("ANT_LDW_OPT") == "1":
    import concourse.bass_utils as _bu
    _orig_rc = _bu.run_command

    def _rc_ldw(argv, **kw):
        argv = ["--enable-ldw-opt=true" if a == "--enable-ldw-opt=false" else a
                for a in argv]
        return _orig_rc(argv, **kw)

    _bu.run_command = _rc_ldw


def _chunks(total, maxn=1024):
    out = []
    c0 = 0
    while c0 < total:
        cn = min(maxn, total - c0)
        out.append((c0, cn))
        c0 += cn
    return out


def build_program(nc, tc, ctx, Tq, Tkv, Dm, Hn):
    """Emit the per-core program. xkv rows are pre-ordered so the first Tq
    tokens are this core's query tokens (attention is permutation-invariant
    over the key axis)."""
    keep = []  # keep tc.tile free-closures alive (GC would release the pools)

    def _tile(shape, dtype, name):
        t, free = tc.tile(shape, dtype, name=name)
        keep.append(free)
        return t, free

    tc._ant_keepalive = keep
    P = 128
    HDl = 64
    assert Dm % P == 0 and Tq % P == 0 and Tkv % P == 0
    DT = Dm // P          # d-tiles
    E3 = 3 * Dm
    PAIRS = Hn // 2       # head pairs; pair = 128 contiguous features
    assert PAIRS * P == Dm and Hn * HDl == Dm
    TBq = Tq // P
    TBkv = Tkv // P
    WE = E3 // P          # qkv_w row tiles
    # eps seen by the post-attention normalize, after folding out the
    # gain/sqrt(T) prefactor (we accumulate raw attn@v).
    eps_av = EPS * math.sqrt(Tkv) / SIGMOID_GAIN

    xkv = nc.dram_tensor("xkv", [Tkv, Dm], F32, kind="ExternalInput").ap()
    qkvw = nc.dram_tensor("qkvw", [E3, Dm], F32, kind="ExternalInput").ap()
    outw = nc.dram_tensor("outw", [Dm, Dm], F32, kind="ExternalInput").ap()
    y = nc.dram_tensor("y", [Tq, Dm], F32, kind="ExternalOutput").ap()

    # ---------------- DRAM scratch ----------------
    dstk = ExitStack()
    dpool = dstk.enter_context(tc.tile_pool(name="dram", bufs=1, space="DRAM"))
    own_dram = dpool.tile([Dm, Dm], BF16, name="own_dram")
    kn_dram = dpool.tile([Tkv, Dm], BF16, name="kn_dram")
    qn_dram = dpool.tile([Tq, Dm], BF16, name="qn_dram")

    # ---------------- persistent SBUF tensors ----------------
    knT, _ = _tile([P, PAIRS * Tkv], BF16, "knT")    # [hd(2 heads), s]
    qnT, _ = _tile([P, PAIRS * Tq], BF16, "qnT")     # [hd(2 heads), t]
    vbig, _ = _tile([P, TBkv * Dm], BF16, "vbig")    # natural [s, e]
    mag8, _ = _tile([P, max(TBq, 2)], F32, "mag8")   # sqrt(||x||^2*HD/D)
    ownT, _ = _tile([P, DT * Dm], BF16, "ownT")      # out_w normalized^T
    avnat, _ = _tile([P, TBq * Dm], BF16, "avnat")   # attn-out natural
    ident, _ = _tile([P, P], BF16, "ident")          # PE-transpose identity
    make_identity(nc, ident)

    # ---------------- phase W + X + A (scoped) ----------------
    wxa = ExitStack()
    wnT, free_wnT = _tile([P, DT * E3], BF16, "wnT")
    xkvT, free_xkvT = _tile([P, DT * Tkv], BF16, "xkvT")
    wstage = wxa.enter_context(tc.tile_pool(name="wstage", bufs=6))
    sqpool = wxa.enter_context(tc.tile_pool(name="sqpool", bufs=4))
    small = wxa.enter_context(tc.tile_pool(name="small", bufs=24))
    nstage = wxa.enter_context(tc.tile_pool(name="nstage", bufs=6))
    psA = wxa.enter_context(tc.tile_pool(name="psA", bufs=2, space="PSUM"))
    psW = wxa.enter_context(tc.tile_pool(name="psW", bufs=2, space="PSUM"))

    def pe_transpose_cols(src, dst_big, cols, stride, base):
        """PE-transpose src [P, DT*P] column blocks into dst_big where block
        dt lands at dst_big[:, dt*stride + base : +cols]."""
        for dt in range(DT):
            ptw = psW.tile([P, P], BF16, name="ptw", tag="ptw")
            nc.tensor.transpose(ptw, src[:, dt * P:(dt + 1) * P], ident)
            nc.vector.tensor_copy(
                dst_big[:, dt * stride + base: dt * stride + base + cols], ptw)

    def normalize_w(we):
        """qkv_w row-tile we -> bf16 rows/(||row||+eps), PE-transposed into
        wnT."""
        wst = wstage.tile([P, Dm], F32, name="wst", tag="wst")
        nc.scalar.dma_start(wst, qkvw[we * P:(we + 1) * P, :])
        wsq = sqpool.tile([P, Dm], BF16, name="wsq", tag="sq")
        ssw = small.tile([P, 1], F32, name="ssw", tag="s1")
        nc.scalar.activation(wsq, wst, AF.Square, accum_out=ssw)
        sw = small.tile([P, 1], F32, name="sw", tag="s1")
        nc.scalar.activation(sw, ssw, AF.Sqrt)
        swe = small.tile([P, 1], F32, name="swe", tag="s1")
        nc.vector.tensor_scalar_add(swe, sw, EPS)
        rw = small.tile([P, 1], F32, name="rw", tag="s1")
        nc.vector.reciprocal(rw, swe)
        wnb = nstage.tile([P, Dm], BF16, name="wnb", tag="nst")
        nc.vector.tensor_scalar_mul(wnb, wst, rw)
        pe_transpose_cols(wnb, wnT, P, E3, we * P)

    def load_x(ti):
        """x token-block ti: magnitude, bf16 cast, PE-transpose into xkvT."""
        xst = wstage.tile([P, Dm], F32, name="xst", tag="wst")
        nc.sync.dma_start(xst, xkv[ti * P:(ti + 1) * P, :])
        if ti < TBq:
            xsq = sqpool.tile([P, Dm], BF16, name="xsq", tag="sq")
            ssx = small.tile([P, 1], F32, name="ssx", tag="s1")
            nc.scalar.activation(xsq, xst, AF.Square, accum_out=ssx)
            nc.scalar.activation(mag8[:, ti:ti + 1], ssx, AF.Sqrt,
                                 scale=float(HDl) / float(Dm))
        xbf = nstage.tile([P, Dm], BF16, name="xbf", tag="nst")
        nc.vector.tensor_copy(xbf, xst)
        pe_transpose_cols(xbf, xkvT, P, Tkv, ti * P)

    # interleave x blocks with k/v weight rows (rows Dm..3Dm); phase A's kv
    # loop needs all kv weight tiles + per-ti x tiles
    for i in range(max(TBkv, 2 * DT)):
        if i < TBkv:
            load_x(i)
        if i < 2 * DT:
            normalize_w(DT + i)
    for we in range(DT):     # q weight rows last (q loop runs after kv loop)
        normalize_w(we)

    # out-projection weights: normalize -> DRAM bounce -> xbar transpose
    # (only needed by phase C; uses idle DMA capacity during A/B)
    for we in range(DT):
        wst = wstage.tile([P, Dm], F32, name="wso", tag="wst")
        nc.scalar.dma_start(wst, outw[we * P:(we + 1) * P, :])
        wsq = sqpool.tile([P, Dm], BF16, name="wsqo", tag="sq")
        ssw = small.tile([P, 1], F32, name="sswo", tag="s1")
        nc.scalar.activation(wsq, wst, AF.Square, accum_out=ssw)
        sw = small.tile([P, 1], F32, name="swo", tag="s1")
        nc.scalar.activation(sw, ssw, AF.Sqrt)
        swe = small.tile([P, 1], F32, name="sweo", tag="s1")
        nc.vector.tensor_scalar_add(swe, sw, EPS)
        rw = small.tile([P, 1], F32, name="rwo", tag="s1")
        nc.vector.reciprocal(rw, swe)
        wnb = nstage.tile([P, Dm], BF16, name="wnbo", tag="nst")
        nc.vector.tensor_scalar_mul(wnb, wst, rw)
        nc.gpsimd.dma_start(own_dram[we * P:(we + 1) * P, :], wnb)
    for dt in range(DT):
        nc.sync.dma_start_transpose(
            ownT[:, dt * Dm:(dt + 1) * Dm],
            own_dram[:, dt * P:(dt + 1) * P])

    # qkv projection + q/k normalization, natural layout
    def qk_normalize(kraw, is_k):
        """kraw: SBUF bf16 [P, Dm] raw q or k; returns normalized bf16 tile."""
        sqk = sqpool.tile([P, Dm], BF16, name="sqk", tag="sq")
        nc.vector.tensor_mul(sqk, kraw, kraw)
        ssk = small.tile([P, Hn], F32, name="ssk", tag="sh")
        nc.vector.tensor_reduce(ssk, sqk.rearrange("p (h d) -> p h d", h=Hn),
                                axis=AX.X, op=ALU.add)
        sk = small.tile([P, Hn], F32, name="sk", tag="sh")
        nc.scalar.activation(sk, ssk, AF.Sqrt)
        ske = small.tile([P, Hn], F32, name="ske", tag="sh")
        if is_k:
            # fold the 1/sqrt(HD) score scale into k: sqrt(HD)/(||k||+eps)
            nc.vector.tensor_scalar(ske, sk, EPS, 1.0 / math.sqrt(HDl),
                                    op0=ALU.add, op1=ALU.mult)
        else:
            nc.vector.tensor_scalar_add(ske, sk, EPS)
        rk = small.tile([P, Hn], F32, name="rk", tag="sh")
        nc.vector.reciprocal(rk, ske)
        knb = nstage.tile([P, Dm], BF16, name="knb", tag="nst")
        nc.vector.tensor_tensor(
            knb.rearrange("p (h d) -> p h d", h=Hn),
            kraw.rearrange("p (h d) -> p h d", h=Hn),
            rk.broadcast_to([P, Hn, HDl]),
            op=ALU.mult)
        return knb

    def emit_q(ti):
        # q for this core's token blocks (first TBq blocks of xkv)
        ps = psA.tile([P, Dm], F32, name="psq", tag="ps")
        for dt in range(DT):
            lhs = xkvT[:, dt * Tkv + ti * P: dt * Tkv + (ti + 1) * P]
            for (c0, cn) in _chunks(Dm, 512):
                nc.tensor.matmul(ps[:, c0:c0 + cn], lhsT=lhs,
                                 rhs=wnT[:, dt * E3 + c0: dt * E3 + c0 + cn],
                                 start=(dt == 0), stop=(dt == DT - 1))
        qraw = sqpool.tile([P, Dm], BF16, name="qraw", tag="kraw")
        nc.scalar.activation(qraw, ps[:, 0:Dm], AF.Copy)
        qnb = qk_normalize(qraw, False)
        nc.gpsimd.dma_start(qn_dram[ti * P:(ti + 1) * P, :], qnb)
        QH = max(TBq // 2, 1)
        if ti % QH == QH - 1:
            h0 = (ti // QH) * QH * P
            hn = QH * P
            for pr in range(PAIRS):
                nc.sync.dma_start_transpose(
                    qnT[:, pr * Tq + h0: pr * Tq + h0 + hn],
                    qn_dram[h0:h0 + hn, pr * P:(pr + 1) * P])

    KQ = max(TBkv // 4, 1)
    qdone = 0
    for ti in range(TBkv):
        # k,v for every token block
        ps = psA.tile([P, 2 * Dm], F32, name="pskv", tag="ps")
        for dt in range(DT):
            lhs = xkvT[:, dt * Tkv + ti * P: dt * Tkv + (ti + 1) * P]
            for (c0, cn) in _chunks(2 * Dm, 512):
                nc.tensor.matmul(ps[:, c0:c0 + cn], lhsT=lhs,
                                 rhs=wnT[:, dt * E3 + Dm + c0: dt * E3 + Dm + c0 + cn],
                                 start=(dt == 0), stop=(dt == DT - 1))
        # evict PSUM quickly (frees the accumulation slot after two ACT copies)
        kraw = sqpool.tile([P, Dm], BF16, name="kraw", tag="kraw")
        nc.scalar.activation(kraw, ps[:, 0:Dm], AF.Copy)
        nc.scalar.activation(vbig[:, ti * Dm:(ti + 1) * Dm], ps[:, Dm:2 * Dm],
                             AF.Copy)
        knb = qk_normalize(kraw, True)
        nc.gpsimd.dma_start(kn_dram[ti * P:(ti + 1) * P, :], knb)
        if ti % KQ == KQ - 1:
            h0 = (ti // KQ) * KQ * P
            hn = KQ * P
            for pr in range(PAIRS):
                nc.sync.dma_start_transpose(
                    knT[:, pr * Tkv + h0: pr * Tkv + h0 + hn],
                    kn_dram[h0:h0 + hn, pr * P:(pr + 1) * P])
        # interleave q token-blocks so the PE stream stays dense into phase B
        qtarget = (ti + 1) * TBq // TBkv
        while qdone < qtarget:
            emit_q(qdone)
            qdone += 1

    wxa.close()
    free_xkvT()
    free_wnT()

    # ---------------- phase B: scores -> sigmoid -> attn @ v ----------------
    # Software-pipelined: scores for unit i+1 are issued to the PE before the
    # attn@v of unit i, so the PE works under each sigmoid instead of stalling
    # in FIFO order behind it. unit = (pair, key-block, head-in-pair).
    avt_big, _ = _tile([P, PAIRS * Tq], BF16, "avt_big")
    bstk = ExitStack()
    psS = bstk.enter_context(tc.tile_pool(name="psS", bufs=3, space="PSUM"))
    psAV = bstk.enter_context(tc.tile_pool(name="psAV", bufs=1, space="PSUM"))
    attnp = bstk.enter_context(tc.tile_pool(name="attnp", bufs=6))

    # unit = (pair, key-block, t-half). One [128, 1024] score tile holds BOTH
    # heads' [128, 512] score blocks side by side: the two K=64 matmuls are
    # emitted adjacently (concurrent in disjoint PE row groups), and ONE
    # FD=1024 sigmoid covers both heads.
    THW = min(512, Tq)
    TH = Tq // THW
    units = [(pr, sb, th) for pr in range(PAIRS) for sb in range(TBkv)
             for th in range(TH)]
    psav_by_pair = {}
    pss_by_unit = {}

    def emit_scores(u):
        pr, sb, th = u
        pss = psS.tile([P, 2 * THW], F32, name="pss", tag="pss")
        pss_by_unit[u] = pss
        for a in (0, 1):
            r0 = a * HDl
            nc.tensor.matmul(
                pss[:, a * THW:(a + 1) * THW],
                lhsT=knT[r0:r0 + HDl, pr * Tkv + sb * P: pr * Tkv + (sb + 1) * P],
                rhs=qnT[r0:r0 + HDl, pr * Tq + th * THW: pr * Tq + (th + 1) * THW],
                start=True, stop=True)

    emit_scores(units[0])
    emit_scores(units[1])
    for i, u in enumerate(units):
        pr, sb, th = u
        if i + 2 < len(units):
            emit_scores(units[i + 2])
        if sb == 0 and th == 0:
            psav_by_pair[pr] = psAV.tile([P, Tq], F32, name="psav", tag="psav")
        psav = psav_by_pair[pr]
        pss = pss_by_unit.pop(u)
        attn = attnp.tile([P, 2 * THW], BF16, name="attn", tag="attn")
        nc.scalar.activation(attn, pss, AF.Sigmoid)
        for a in (0, 1):
            r0 = a * HDl
            nc.tensor.matmul(
                psav[r0:r0 + HDl, th * THW:(th + 1) * THW],
                lhsT=vbig[:, sb * Dm + pr * P + r0: sb * Dm + pr * P + r0 + HDl],
                rhs=attn[:, a * THW:(a + 1) * THW],
                start=(sb == 0), stop=(sb == TBkv - 1),
                skip_group_check=True)
        if sb == TBkv - 1 and th == TH - 1:
            nc.vector.tensor_copy(avt_big[:, pr * Tq:(pr + 1) * Tq], psav)
    bstk.close()

    # ---------------- phase C: normalize + magnitude + out-proj ----------------
    avnT, _ = _tile([P, DT * Tq], BF16, "avnT")
    cstk = ExitStack()
    psO = cstk.enter_context(tc.tile_pool(name="psO", bufs=2, space="PSUM"))
    psT2 = cstk.enter_context(tc.tile_pool(name="psT2", bufs=4, space="PSUM"))
    sqc = cstk.enter_context(tc.tile_pool(name="sqc", bufs=4))
    smallc = cstk.enter_context(tc.tile_pool(name="smallc", bufs=24))
    avnp = cstk.enter_context(tc.tile_pool(name="avnp", bufs=4))
    ypool = cstk.enter_context(tc.tile_pool(name="ypool", bufs=3))

    def c_avT(tb):
        for pr in range(PAIRS):
            ptt = psT2.tile([P, P], BF16, name="ptta", tag="ptt2")
            nc.tensor.transpose(
                ptt, avt_big[:, pr * Tq + tb * P: pr * Tq + (tb + 1) * P], ident)
            nc.scalar.activation(
                avnat[:, tb * Dm + pr * P: tb * Dm + (pr + 1) * P], ptt, AF.Copy)

    def c_norm(tb):
        src = avnat[:, tb * Dm:(tb + 1) * Dm]
        sqa = sqc.tile([P, Dm], BF16, name="sqa", tag="sqa")
        nc.vector.tensor_mul(sqa, src, src)
        ssa = smallc.tile([P, Hn], F32, name="ssa", tag="sh")
        nc.vector.tensor_reduce(ssa, sqa.rearrange("p (h d) -> p h d", h=Hn),
                                axis=AX.X, op=ALU.add)
        sa = smallc.tile([P, Hn], F32, name="sa", tag="sh")
        nc.scalar.activation(sa, ssa, AF.Sqrt)
        sae = smallc.tile([P, Hn], F32, name="sae", tag="sh")
        nc.vector.tensor_scalar_add(sae, sa, eps_av)
        ra = smallc.tile([P, Hn], F32, name="ra", tag="sh")
        nc.vector.reciprocal(ra, sae)
        g = smallc.tile([P, Hn], F32, name="g", tag="sh")
        nc.vector.tensor_scalar_mul(g, ra, mag8[:, tb:tb + 1])
        avn = avnp.tile([P, Dm], BF16, name="avn", tag="avn")
        nc.vector.tensor_tensor(
            avn.rearrange("p (h d) -> p h d", h=Hn),
            src.rearrange("p (h d) -> p h d", h=Hn),
            g.broadcast_to([P, Hn, HDl]),
            op=ALU.mult)
        for dt in range(DT):
            ptt = psT2.tile([P, P], BF16, name="ptt2", tag="ptt2")
            nc.tensor.transpose(ptt, avn[:, dt * P:(dt + 1) * P], ident)
            nc.vector.tensor_copy(
                avnT[:, dt * Tq + tb * P: dt * Tq + (tb + 1) * P], ptt)

    def c_proj(tb):
        pso = psO.tile([P, Dm], F32, name="pso", tag="pso")
        for dt in range(DT):
            lhs = avnT[:, dt * Tq + tb * P: dt * Tq + (tb + 1) * P]
            for (c0, cn) in _chunks(Dm, 512):
                nc.tensor.matmul(pso[:, c0:c0 + cn], lhsT=lhs,
                                 rhs=ownT[:, dt * Dm + c0: dt * Dm + c0 + cn],
                                 start=(dt == 0), stop=(dt == DT - 1))
        ysb = ypool.tile([P, Dm], F32, name="ysb", tag="ysb")
        nc.scalar.activation(ysb, pso, AF.Copy)
        nc.gpsimd.dma_start(y[tb * P:(tb + 1) * P, :], ysb)

    for tb in range(TBq + 2):
        if tb < TBq:
            c_avT(tb)
        if tb >= 1 and tb - 1 < TBq:
            c_norm(tb - 1)
        if tb >= 2:
            c_proj(tb - 2)
    cstk.close()
    dstk.close()


def make_nc(Tq=T // 2, Tkv=T, Dm=D, Hn=H):
    nc = bacc.Bacc("TRN2", target_bir_lowering=False, debug=False,
                   num_devices=N_CORES)
    with ExitStack() as ctx:
        with tile.TileContext(nc) as tc:
            build_program(nc, tc, ctx, Tq, Tkv, Dm, Hn)
    nc.compile()
    return nc


_CACHED_NC = None


def _get_nc():
    global _CACHED_NC
    if _CACHED_NC is None:
        _CACHED_NC = make_nc()
    return _CACHED_NC


def _shard_inputs(x, qkv_w, out_w):
    Tq = T // 2
    x = np.asarray(x, dtype=np.float32)
    qkv_w = np.ascontiguousarray(np.asarray(qkv_w, dtype=np.float32))
    out_w = np.ascontiguousarray(np.asarray(out_w, dtype=np.float32))
    in_maps = []
    for core in range(N_CORES):
        b, half = core // 2, core % 2
        own = x[b, half * Tq:(half + 1) * Tq]
        other = x[b, (1 - half) * Tq:(2 - half) * Tq]
        xkv = np.ascontiguousarray(np.concatenate([own, other], axis=0))
        in_maps.append({"xkv": xkv, "qkvw": qkv_w, "outw": out_w})
    return in_maps


def run(x, qkv_w, out_w, trace=False, trace_cores=None):
    nc = _get_nc()
    in_maps = _shard_inputs(x, qkv_w, out_w)
    res = run_bass_kernel_spmd(nc, in_maps, list(range(N_CORES)),
                               trace=trace, trace_cores=trace_cores)
    Tq = T // 2
    y = np.empty((B, T, D), np.float32)
    for core, r in enumerate(res.results):
        b, half = core // 2, core % 2
        y[b, half * Tq:(half + 1) * Tq] = r["y"]
    return y, res


def kernel(x, qkv_w, out_w):
    y, _ = run(x, qkv_w, out_w, trace=False)
    return y


# revision 29
# speedup vs baseline: 1.3130x; 1.0067x over previous
"""Trainium2 Bass kernel for nn_Attention_4363686773373.

Sigmoid attention with magnitude-preserving (weight-normalized) projections.

Sharding: data-parallel over (batch, T-half) -> 8 shards on 8 NeuronCores.
Each core computes q for its 1024 tokens and k,v for the full 2048 tokens of
its batch (k/v recomputed on both cores of a batch; no collectives). Each
core's xkv rows are pre-ordered so its query tokens come first (attention is
permutation-invariant over the key axis), keeping the program SPMD-uniform.

Per-core dataflow (heavy matmuls in bf16 with fp32 PSUM accumulation):
  W/X: row-normalize qkv_w on device and PE-transpose it (and bf16-cast x)
     into [d, .] layouts; out_w is normalized and bounced through DRAM with
     large DMA-xbar transposes (only needed by the out-projection); per-token
     ||x|| comes from an ACT square+accumulate.
  A: qkv projection in natural [t, e] layout (lhsT = xT tiles), fast PSUM
     eviction through ACT copies, q/k cosine-normalization along head_dim via
     free-dim reduces, then DRAM-bounce transposes to [head_dim, t] layout.
     q token-blocks are interleaved into the k/v loop to keep the PE dense.
  B: unit = (head-pair, key-block, t-half). Both heads' K=64 score matmuls
     land side by side in one [128, 1024] PSUM tile (adjacent issue -> they
     run concurrently in disjoint PE row groups), ONE FD=1024 sigmoid on the
     scalar engine converts scores to bf16 attn weights, and attn^T @ v
     accumulates per pair. Score tiles are triple-buffered and issued two
     units ahead so the PE never stalls inside a sigmoid (keeps the PE HAM
     clock warm - the single biggest performance lever observed).
  C: software-pipelined per token-block: PE-transpose attn-out to natural
     layout, normalize per (token, head), scale by token magnitude,
     PE-transpose back, out-projection, store.

DMA rings: sync/scalar = loads + xbar transposes, gpsimd = scratch writes +
output stores. ACT table sets: sqrt-set phases strictly precede the sigmoid
phase and the final sqrt-set phase (2 table switches total).
"""

import math
from contextlib import ExitStack

import numpy as np

import concourse.bass as bass
import concourse.tile as tile
from concourse import bacc, mybir
from concourse.bass_utils import run_bass_kernel_spmd
from concourse.masks import make_identity

# Problem shapes (hardcoded per harness contract)
B, T, D, H = 4, 2048, 768, 12
HD = D // H  # 64
EPS = 1e-4
SIGMOID_GAIN = 1.8402
N_CORES = 8

F32 = mybir.dt.float32
BF16 = mybir.dt.bfloat16
AF = mybir.ActivationFunctionType
ALU = mybir.AluOpType
AX = mybir.AxisListType


def _ensure_axon_hooks():
    """This image's antenv lacks axon_hooks; reconstruct it so trace=True
    (NTFF profiling) works instead of crashing on import."""
    try:
        import antenv.axon_hooks  # noqa: F401
        return
    except ImportError:
        pass
    import sys
    import types
    try:
        import antenv
    except ImportError:
        return
    mod = types.ModuleType("antenv.axon_hooks")
    _hook = [None]
    mod.set_axon_ntff_profile_hook = lambda h: _hook.__setitem__(0, h)
    mod.get_axon_ntff_profile_hook = lambda: _hook[0]
    sys.modules["antenv.axon_hooks"] = mod
    antenv.axon_hooks = mod
    try:
        from trn_agent_boot.trn_boot import _ntff_profile_via_ctypes
        mod.set_axon_ntff_profile_hook(
            _ntff_profile_via_ctypes('/opt/axon/libaxon_pjrt.so'))
    except Exception:
        pass


_ensure_axon_hooks()

if __import__("os").environ.get("ANT_LDW_OPT") == "1":
    import concourse.bass_utils as _bu
    _orig_rc = _bu.run_command

    def _rc_ldw(argv, **kw):
        argv = ["--enable-ldw-opt=true" if a == "--enable-ldw-opt=false" else a
                for a in argv]
        return _orig_rc(argv, **kw)

    _bu.run_command = _rc_ldw


def _chunks(total, maxn=1024):
    out = []
    c0 = 0
    while c0 < total:
        cn = min(maxn, total - c0)
        out.append((c0, cn))
        c0 += cn
    return out


def build_program(nc, tc, ctx, Tq, Tkv, Dm, Hn):
    """Emit the per-core program. xkv rows are pre-ordered so the first Tq
    tokens are this core's query tokens (attention is permutation-invariant
    over the key axis)."""
    keep = []  # keep tc.tile free-closures alive (GC would release the pools)

    def _tile(shape, dtype, name):
        t, free = tc.tile(shape, dtype, name=name)
        keep.append(free)
        return t, free

    tc._ant_keepalive = keep
    P = 128
    HDl = 64
    assert Dm % P == 0 and Tq % P == 0 and Tkv % P == 0
    DT = Dm // P          # d-tiles
    E3 = 3 * Dm
    PAIRS = Hn // 2       # head pairs; pair = 128 contiguous features
    assert PAIRS * P == Dm and Hn * HDl == Dm
    TBq = Tq // P
    TBkv = Tkv // P
    WE = E3 // P          # qkv_w row tiles
    # eps seen by the post-attention normalize, after folding out the
    # gain/sqrt(T) prefactor (we accumulate raw attn@v).
    eps_av = EPS * math.sqrt(Tkv) / SIGMOID_GAIN

    xkv = nc.dram_tensor("xkv", [Tkv, Dm], F32, kind="ExternalInput").ap()
    qkvw = nc.dram_tensor("qkvw", [E3, Dm], F32, kind="ExternalInput").ap()
    outw = nc.dram_tensor("outw", [Dm, Dm], F32, kind="ExternalInput").ap()
    y = nc.dram_tensor("y", [Tq, Dm], F32, kind="ExternalOutput").ap()

    # ---------------- DRAM scratch ----------------
    dstk = ExitStack()
    dpool = dstk.enter_context(tc.tile_pool(name="dram", bufs=1, space="DRAM"))
    own_dram = dpool.tile([Dm, Dm], BF16, name="own_dram")
    kn_dram = dpool.tile([Tkv, Dm], BF16, name="kn_dram")
    qn_dram = dpool.tile([Tq, Dm], BF16, name="qn_dram")

    # ---------------- persistent SBUF tensors ----------------
    knT, _ = _tile([P, PAIRS * Tkv], BF16, "knT")    # [hd(2 heads), s]
    qnT, _ = _tile([P, PAIRS * Tq], BF16, "qnT")     # [hd(2 heads), t]
    vbig, _ = _tile([P, TBkv * Dm], BF16, "vbig")    # natural [s, e]
    mag8, _ = _tile([P, max(TBq, 2)], F32, "mag8")   # sqrt(||x||^2*HD/D)
    ownT, _ = _tile([P, DT * Dm], BF16, "ownT")      # out_w normalized^T
    avnat, _ = _tile([P, TBq * Dm], BF16, "avnat")   # attn-out natural
    ident, _ = _tile([P, P], BF16, "ident")          # PE-transpose identity
    make_identity(nc, ident)

    # ---------------- phase W + X + A (scoped) ----------------
    wxa = ExitStack()
    wnT, free_wnT = _tile([P, DT * E3], BF16, "wnT")
    xkvT, free_xkvT = _tile([P, DT * Tkv], BF16, "xkvT")
    wstage = wxa.enter_context(tc.tile_pool(name="wstage", bufs=6))
    sqpool = wxa.enter_context(tc.tile_pool(name="sqpool", bufs=4))
    small = wxa.enter_context(tc.tile_pool(name="small", bufs=24))
    nstage = wxa.enter_context(tc.tile_pool(name="nstage", bufs=6))
    psA = wxa.enter_context(tc.tile_pool(name="psA", bufs=2, space="PSUM"))
    psW = wxa.enter_context(tc.tile_pool(name="psW", bufs=2, space="PSUM"))

    def pe_transpose_cols(src, dst_big, cols, stride, base):
        """PE-transpose src [P, DT*P] column blocks into dst_big where block
        dt lands at dst_big[:, dt*stride + base : +cols]."""
        for dt in range(DT):
            ptw = psW.tile([P, P], BF16, name="ptw", tag="ptw")
            nc.tensor.transpose(ptw, src[:, dt * P:(dt + 1) * P], ident)
            nc.vector.tensor_copy(
                dst_big[:, dt * stride + base: dt * stride + base + cols], ptw)

    def normalize_w(we):
        """qkv_w row-tile we -> bf16 rows/(||row||+eps), PE-transposed into
        wnT."""
        wst = wstage.tile([P, Dm], F32, name="wst", tag="wst")
        nc.scalar.dma_start(wst, qkvw[we * P:(we + 1) * P, :])
        wsq = sqpool.tile([P, Dm], BF16, name="wsq", tag="sq")
        ssw = small.tile([P, 1], F32, name="ssw", tag="s1")
        nc.scalar.activation(wsq, wst, AF.Square, accum_out=ssw)
        sw = small.tile([P, 1], F32, name="sw", tag="s1")
        nc.scalar.activation(sw, ssw, AF.Sqrt)
        swe = small.tile([P, 1], F32, name="swe", tag="s1")
        nc.vector.tensor_scalar_add(swe, sw, EPS)
        rw = small.tile([P, 1], F32, name="rw", tag="s1")
        nc.vector.reciprocal(rw, swe)
        wnb = nstage.tile([P, Dm], BF16, name="wnb", tag="nst")
        nc.vector.tensor_scalar_mul(wnb, wst, rw)
        pe_transpose_cols(wnb, wnT, P, E3, we * P)

    def load_x(ti):
        """x token-block ti: magnitude, bf16 cast, PE-transpose into xkvT."""
        xst = wstage.tile([P, Dm], F32, name="xst", tag="wst")
        nc.sync.dma_start(xst, xkv[ti * P:(ti + 1) * P, :])
        if ti < TBq:
            xsq = sqpool.tile([P, Dm], BF16, name="xsq", tag="sq")
            ssx = small.tile([P, 1], F32, name="ssx", tag="s1")
            nc.scalar.activation(xsq, xst, AF.Square, accum_out=ssx)
            nc.scalar.activation(mag8[:, ti:ti + 1], ssx, AF.Sqrt,
                                 scale=float(HDl) / float(Dm))
        xbf = nstage.tile([P, Dm], BF16, name="xbf", tag="nst")
        nc.vector.tensor_copy(xbf, xst)
        pe_transpose_cols(xbf, xkvT, P, Tkv, ti * P)

    # interleave x blocks with k/v weight rows (rows Dm..3Dm); phase A's kv
    # loop needs all kv weight tiles + per-ti x tiles
    for i in range(max(TBkv, 2 * DT)):
        if i < TBkv:
            load_x(i)
        if i < 2 * DT:
            normalize_w(DT + i)
    for we in range(DT):     # q weight rows last (q loop runs after kv loop)
        normalize_w(we)

    # out-projection weights: normalize -> DRAM bounce -> xbar transpose
    # (only needed by phase C; uses idle DMA capacity during A/B)
    for we in range(DT):
        wst = wstage.tile([P, Dm], F32, name="wso", tag="wst")
        nc.scalar.dma_start(wst, outw[we * P:(we + 1) * P, :])
        wsq = sqpool.tile([P, Dm], BF16, name="wsqo", tag="sq")
        ssw = small.tile([P, 1], F32, name="sswo", tag="s1")
        nc.scalar.activation(wsq, wst, AF.Square, accum_out=ssw)
        sw = small.tile([P, 1], F32, name="swo", tag="s1")
        nc.scalar.activation(sw, ssw, AF.Sqrt)
        swe = small.tile([P, 1], F32, name="sweo", tag="s1")
        nc.vector.tensor_scalar_add(swe, sw, EPS)
        rw = small.tile([P, 1], F32, name="rwo", tag="s1")
        nc.vector.reciprocal(rw, swe)
        wnb = nstage.tile([P, Dm], BF16, name="wnbo", tag="nst")
        nc.vector.tensor_scalar_mul(wnb, wst, rw)
        nc.gpsimd.dma_start(own_dram[we * P:(we + 1) * P, :], wnb)
    for dt in range(DT):
        nc.sync.dma_start_transpose(
            ownT[:, dt * Dm:(dt + 1) * Dm],
            own_dram[:, dt * P:(dt + 1) * P])

    # qkv projection + q/k normalization, natural layout
    def qk_normalize(kraw, is_k):
        """kraw: SBUF bf16 [P, Dm] raw q or k; returns normalized bf16 tile."""
        sqk = sqpool.tile([P, Dm], BF16, name="sqk", tag="sq")
        nc.vector.tensor_mul(sqk, kraw, kraw)
        ssk = small.tile([P, Hn], F32, name="ssk", tag="sh")
        nc.vector.tensor_reduce(ssk, sqk.rearrange("p (h d) -> p h d", h=Hn),
                                axis=AX.X, op=ALU.add)
        sk = small.tile([P, Hn], F32, name="sk", tag="sh")
        nc.scalar.activation(sk, ssk, AF.Sqrt)
        ske = small.tile([P, Hn], F32, name="ske", tag="sh")
        if is_k:
            # fold the 1/sqrt(HD) score scale into k: sqrt(HD)/(||k||+eps)
            nc.vector.tensor_scalar(ske, sk, EPS, 1.0 / math.sqrt(HDl),
                                    op0=ALU.add, op1=ALU.mult)
        else:
            nc.vector.tensor_scalar_add(ske, sk, EPS)
        rk = small.tile([P, Hn], F32, name="rk", tag="sh")
        nc.vector.reciprocal(rk, ske)
        knb = nstage.tile([P, Dm], BF16, name="knb", tag="nst")
        nc.vector.tensor_tensor(
            knb.rearrange("p (h d) -> p h d", h=Hn),
            kraw.rearrange("p (h d) -> p h d", h=Hn),
            rk.broadcast_to([P, Hn, HDl]),
            op=ALU.mult)
        return knb

    def emit_q(ti):
        # q for this core's token blocks (first TBq blocks of xkv)
        ps = psA.tile([P, Dm], F32, name="psq", tag="ps")
        for dt in range(DT):
            lhs = xkvT[:, dt * Tkv + ti * P: dt * Tkv + (ti + 1) * P]
            for (c0, cn) in _chunks(Dm, 512):
                nc.tensor.matmul(ps[:, c0:c0 + cn], lhsT=lhs,
                                 rhs=wnT[:, dt * E3 + c0: dt * E3 + c0 + cn],
                                 start=(dt == 0), stop=(dt == DT - 1))
        qraw = sqpool.tile([P, Dm], BF16, name="qraw", tag="kraw")
        nc.scalar.activation(qraw, ps[:, 0:Dm], AF.Copy)
        qnb = qk_normalize(qraw, False)
        nc.gpsimd.dma_start(qn_dram[ti * P:(ti + 1) * P, :], qnb)
        QH = max(TBq // 2, 1)
        if ti % QH == QH - 1:
            h0 = (ti // QH) * QH * P
            hn = QH * P
            for pr in range(PAIRS):
                nc.sync.dma_start_transpose(
                    qnT[:, pr * Tq + h0: pr * Tq + h0 + hn],
                    qn_dram[h0:h0 + hn, pr * P:(pr + 1) * P])

    KQ = max(TBkv // 4, 1)
    qdone = 0
    for ti in range(TBkv):
        # k,v for every token block
        ps = psA.tile([P, 2 * Dm], F32, name="pskv", tag="ps")
        for dt in range(DT):
            lhs = xkvT[:, dt * Tkv + ti * P: dt * Tkv + (ti + 1) * P]
            for (c0, cn) in _chunks(2 * Dm, 512):
                nc.tensor.matmul(ps[:, c0:c0 + cn], lhsT=lhs,
                                 rhs=wnT[:, dt * E3 + Dm + c0: dt * E3 + Dm + c0 + cn],
                                 start=(dt == 0), stop=(dt == DT - 1))
        # evict PSUM quickly (frees the accumulation slot after two ACT copies)
        kraw = sqpool.tile([P, Dm], BF16, name="kraw", tag="kraw")
        nc.scalar.activation(kraw, ps[:, 0:Dm], AF.Copy)
        nc.scalar.activation(vbig[:, ti * Dm:(ti + 1) * Dm], ps[:, Dm:2 * Dm],
                             AF.Copy)
        knb = qk_normalize(kraw, True)
        nc.gpsimd.dma_start(kn_dram[ti * P:(ti + 1) * P, :], knb)
        if ti % KQ == KQ - 1:
            h0 = (ti // KQ) * KQ * P
            hn = KQ * P
            for pr in range(PAIRS):
                nc.sync.dma_start_transpose(
                    knT[:, pr * Tkv + h0: pr * Tkv + h0 + hn],
                    kn_dram[h0:h0 + hn, pr * P:(pr + 1) * P])
        # interleave q token-blocks so the PE stream stays dense into phase B
        qtarget = (ti + 1) * TBq // TBkv
        while qdone < qtarget:
            emit_q(qdone)
            qdone += 1

    wxa.close()
    free_xkvT()
    free_wnT()

    # ---------------- phase B: scores -> sigmoid -> attn @ v ----------------
    # Software-pipelined: scores for unit i+1 are issued to the PE before the
    # attn@v of unit i, so the PE works under each sigmoid instead of stalling
    # in FIFO order behind it. unit = (pair, key-block, head-in-pair).
    avt_big, _ = _tile([P, PAIRS * Tq], BF16, "avt_big")
    bstk = ExitStack()
    psS = bstk.enter_context(tc.tile_pool(name="psS", bufs=3, space="PSUM"))
    psAV = bstk.enter_context(tc.tile_pool(name="psAV", bufs=1, space="PSUM"))
    attnp = bstk.enter_context(tc.tile_pool(name="attnp", bufs=6))

    # unit = (pair, key-block, t-half). One [128, 1024] score tile holds BOTH
    # heads' [128, 512] score blocks side by side: the two K=64 matmuls are
    # emitted adjacently (concurrent in disjoint PE row groups), and ONE
    # FD=1024 sigmoid covers both heads.
    THW = min(512, Tq)
    TH = Tq // THW
    units = [(pr, sb, th) for pr in range(PAIRS) for sb in range(TBkv)
             for th in range(TH)]
    psav_by_pair = {}
    pss_by_unit = {}

    def emit_scores(u):
        pr, sb, th = u
        pss = psS.tile([P, 2 * THW], F32, name="pss", tag="pss")
        pss_by_unit[u] = pss
        for a in (0, 1):
            r0 = a * HDl
            nc.tensor.matmul(
                pss[:, a * THW:(a + 1) * THW],
                lhsT=knT[r0:r0 + HDl, pr * Tkv + sb * P: pr * Tkv + (sb + 1) * P],
                rhs=qnT[r0:r0 + HDl, pr * Tq + th * THW: pr * Tq + (th + 1) * THW],
                start=True, stop=True)

    emit_scores(units[0])
    emit_scores(units[1])
    for i, u in enumerate(units):
        pr, sb, th = u
        if i + 2 < len(units):
            emit_scores(units[i + 2])
        if sb == 0 and th == 0:
            psav_by_pair[pr] = psAV.tile([P, Tq], F32, name="psav", tag="psav")
        psav = psav_by_pair[pr]
        pss = pss_by_unit.pop(u)
        attn = attnp.tile([P, 2 * THW], BF16, name="attn", tag="attn")
        nc.scalar.activation(attn, pss, AF.Sigmoid)
        for a in (0, 1):
            r0 = a * HDl
            nc.tensor.matmul(
                psav[r0:r0 + HDl, th * THW:(th + 1) * THW],
                lhsT=vbig[:, sb * Dm + pr * P + r0: sb * Dm + pr * P + r0 + HDl],
                rhs=attn[:, a * THW:(a + 1) * THW],
                start=(sb == 0), stop=(sb == TBkv - 1),
                skip_group_check=True)
        if sb == TBkv - 1 and th == TH - 1:
            nc.vector.tensor_copy(avt_big[:, pr * Tq:(pr + 1) * Tq], psav)
    bstk.close()

    # ---------------- phase C: normalize + magnitude + out-proj ----------------
    avnT, _ = _tile([P, DT * Tq], BF16, "avnT")
    cstk = ExitStack()
    psO = cstk.enter_context(tc.tile_pool(name="psO", bufs=2, space="PSUM"))
    psT2 = cstk.enter_context(tc.tile_pool(name="psT2", bufs=4, space="PSUM"))
    sqc = cstk.enter_context(tc.tile_pool(name="sqc", bufs=4))
    smallc = cstk.enter_context(tc.tile_pool(name="smallc", bufs=24))
    avnp = cstk.enter_context(tc.tile_pool(name="avnp", bufs=4))
    ypool = cstk.enter_context(tc.tile_pool(name="ypool", bufs=3))

    def c_avT(tb):
        for pr in range(PAIRS):
            ptt = psT2.tile([P, P], BF16, name="ptta", tag="ptt2")
            nc.tensor.transpose(
                ptt, avt_big[:, pr * Tq + tb * P: pr * Tq + (tb + 1) * P], ident)
            nc.scalar.activation(
                avnat[:, tb * Dm + pr * P: tb * Dm + (pr + 1) * P], ptt, AF.Copy)

    def c_norm(tb):
        src = avnat[:, tb * Dm:(tb + 1) * Dm]
        sqa = sqc.tile([P, Dm], BF16, name="sqa", tag="sqa")
        nc.vector.tensor_mul(sqa, src, src)
        ssa = smallc.tile([P, Hn], F32, name="ssa", tag="sh")
        nc.vector.tensor_reduce(ssa, sqa.rearrange("p (h d) -> p h d", h=Hn),
                                axis=AX.X, op=ALU.add)
        sa = smallc.tile([P, Hn], F32, name="sa", tag="sh")
        nc.scalar.activation(sa, ssa, AF.Sqrt)
        sae = smallc.tile([P, Hn], F32, name="sae", tag="sh")
        nc.vector.tensor_scalar_add(sae, sa, eps_av)
        ra = smallc.tile([P, Hn], F32, name="ra", tag="sh")
        nc.vector.reciprocal(ra, sae)
        g = smallc.tile([P, Hn], F32, name="g", tag="sh")
        nc.vector.tensor_scalar_mul(g, ra, mag8[:, tb:tb + 1])
        avn = avnp.tile([P, Dm], BF16, name="avn", tag="avn")
        nc.vector.tensor_tensor(
            avn.rearrange("p (h d) -> p h d", h=Hn),
            src.rearrange("p (h d) -> p h d", h=Hn),
            g.broadcast_to([P, Hn, HDl]),
            op=ALU.mult)
        for dt in range(DT):
            ptt = psT2.tile([P, P], BF16, name="ptt2", tag="ptt2")
            nc.tensor.transpose(ptt, avn[:, dt * P:(dt + 1) * P], ident)
            nc.vector.tensor_copy(
                avnT[:, dt * Tq + tb * P: dt * Tq + (tb + 1) * P], ptt)

    def c_proj(tb):
        pso = psO.tile([P, Dm], F32, name="pso", tag="pso")
        for dt in range(DT):
            lhs = avnT[:, dt * Tq + tb * P: dt * Tq + (tb + 1) * P]
            for (c0, cn) in _chunks(Dm, 512):
                nc.tensor.matmul(pso[:, c0:c0 + cn], lhsT=lhs,
                                 rhs=ownT[:, dt * Dm + c0: dt * Dm + c0 + cn],
                                 start=(dt == 0), stop=(dt == DT - 1))
        ysb = ypool.tile([P, Dm], F32, name="ysb", tag="ysb")
        nc.scalar.activation(ysb, pso, AF.Copy)
        nc.gpsimd.dma_start(y[tb * P:(tb + 1) * P, :], ysb)

    for tb in range(TBq + 2):
        if tb < TBq:
            c_avT(tb)
        if tb >= 1 and tb - 1 < TBq:
            c_norm(tb - 1)
        if tb >= 2:
            c_proj(tb - 2)
    cstk.close()
    dstk.close()


def make_nc(Tq=T // 2, Tkv=T, Dm=D, Hn=H):
    nc = bacc.Bacc("TRN2", target_bir_lowering=False, debug=False,
                   num_devices=N_CORES)
    with ExitStack() as ctx:
        with tile.TileContext(nc) as tc:
            build_program(nc, tc, ctx, Tq, Tkv, Dm, Hn)
    nc.compile()
    return nc


_CACHED_NC = None


def _get_nc():
    global _CACHED_NC
    if _CACHED_NC is None:
        _CACHED_NC = make_nc()
    return _CACHED_NC


def _shard_inputs(x, qkv_w, out_w):
    Tq = T // 2
    x = np.asarray(x, dtype=np.float32)
    qkv_w = np.ascontiguousarray(np.asarray(qkv_w, dtype=np.float32))
    out_w = np.ascontiguousarray(np.asarray(out_w, dtype=np.float32))
    in_maps = []
    for core in range(N_CORES):
        b, half = core // 2, core % 2
        own = x[b, half * Tq:(half + 1) * Tq]
        other = x[b, (1 - half) * Tq:(2 - half) * Tq]
        xkv = np.ascontiguousarray(np.concatenate([own, other], axis=0))
        in_maps.append({"xkv": xkv, "qkvw": qkv_w, "outw": out_w})
    return in_maps


def run(x, qkv_w, out_w, trace=False, trace_cores=None):
    nc = _get_nc()
    in_maps = _shard_inputs(x, qkv_w, out_w)
    res = run_bass_kernel_spmd(nc, in_maps, list(range(N_CORES)),
                               trace=trace, trace_cores=trace_cores)
    Tq = T // 2
    y = np.empty((B, T, D), np.float32)
    for core, r in enumerate(res.results):
        b, half = core // 2, core % 2
        y[b, half * Tq:(half + 1) * Tq] = r["y"]
    return y, res


def kernel(x, qkv_w, out_w):
    y, _ = run(x, qkv_w, out_w, trace=False)
    return y
